# revision 53
# baseline (speedup 1.0000x reference)
"""Causal self-attention (S=2048, B=4, D=768, H=12, Hd=64) on 8 TRN2 cores.

Sharding: core c -> (batch b = c//2, head-group hg = c%2).  Each core computes
full-seq attention for one batch element and 6 of the 12 heads plus the Wo
projection restricted to its heads' columns; the host sums the two head-group
partials per batch (fp16 partials -> fp32 on host).

v2 design (vs baseline):
  - g-major loop: per query-group g (512 q), attend over all 3 head-pair
    tiles, then normalize + Wo-project + store that group's y while group
    g+1 attends.  No end-of-kernel stall.
  - softmax exp split across engines: diagonal blocks exact exp on Scalar
    (+ narrow [128]-col mask multiply on GpSimd); off-diagonal blocks split
    between DVE (one-op Schraudolph fp16-bit exp via tensor_scalar ->int16)
    and Scalar.
  - deferred normalization: denominators ride the AV matmul (ones column,
    M=65); per (et,g): 2 scalar copies -> reciprocal_approx_fast (DVE) ->
    K=2 broadcast matmul (f32r) -> fused copy*normalize into attnT (DVE).
  - host pre-rearranged weights (contiguous DMA), x on sync queue, weights
    on scalar queue, fp16 y output.
"""

import numpy as np

S = 2048
B = 4
D = 768
HD = 64
H = 6          # heads per core
E = H * HD     # 384
ND = D // 128  # 6
NE = E // 128  # 3
NT = S // 128  # 16
NG = S // 512  # 4

# Schraudolph fp16-bits exp: bits16 = round(raw_score*A + Bc); see exp_precision.py
SCH_A = 184.66494318  # 0.125 * 1024 / ln(2)
SCH_B = 15316.0       # 15360 + c_adj (c_adj=-44)

_cached = None


def _build():
    import os

    import concourse.mybir as mybir
    import concourse.tile as tile
    from concourse import bacc

    f32 = mybir.dt.float32
    f32r = mybir.dt.float32r
    f16 = mybir.dt.float16
    i16 = mybir.dt.int16
    Alu = mybir.AluOpType
    Act = mybir.ActivationFunctionType

    _no_sch = os.environ.get("K_NO_SCH", "0") == "1"
    _no_fused = os.environ.get("K_NO_FUSED", "0") == "1"

    nc = bacc.Bacc("TRN2")

    xT_d = nc.dram_tensor("xT", [D, S], f16, kind="ExternalInput")
    wq_d = nc.dram_tensor("wq", [128, ND, E], f16, kind="ExternalInput")
    wk_d = nc.dram_tensor("wk", [128, ND, E], f16, kind="ExternalInput")
    wv_d = nc.dram_tensor("wv", [128, ND, E], f16, kind="ExternalInput")
    wo_d = nc.dram_tensor("wo", [128, NE, D], f16, kind="ExternalInput")
    maskt_d = nc.dram_tensor("maskt", [128, 128], f16, kind="ExternalInput")
    y_d = nc.dram_tensor("y", [S, D], f16, kind="ExternalOutput")

    with tile.TileContext(nc) as tc:
        with (
            tc.tile_pool(name="xt", bufs=6) as xt_pool,
            tc.tile_pool(name="at", bufs=3) as at_pool,
            tc.tile_pool(name="w", bufs=1) as w_pool,
            tc.tile_pool(name="qk", bufs=6) as qk_pool,
            tc.tile_pool(name="pt", bufs=6) as pt_pool,
            tc.tile_pool(name="dn", bufs=6) as dn_pool,
            tc.tile_pool(name="rbp", bufs=6) as rbs_pool,
            tc.tile_pool(name="y", bufs=4) as y_pool,
            tc.tile_pool(name="ps2", bufs=2, space="PSUM") as ps2_pool,
            tc.tile_pool(name="po", bufs=4, space="PSUM") as po_pool,
        ):
            wq = w_pool.tile([128, ND, E], f16, tag="wq")
            wk = w_pool.tile([128, ND, E], f16, tag="wk")
            wv = w_pool.tile([128, ND, E], f16, tag="wv")
            wo = w_pool.tile([128, NE, D], f16, tag="wo")
            maskt = w_pool.tile([128, 128], f16, tag="maskt")
            vaug = w_pool.tile([128, NT, H, 65], f16, tag="vaug")

            # x chunks split across both DMA queues (ch-major so t=0..3 land
            # first); wv leads the scalar queue so project_v can start early
            xT = [
                xt_pool.tile([128, S], f16, tag="xt", name=f"xT{d}")
                for d in range(ND)
            ]
            nc.sync.dma_start(wv[:, 0:3, :], wv_d[:, 0:3, :])
            nc.scalar.dma_start(wv[:, 3:6, :], wv_d[:, 3:6, :])

            def load_x(ch, d, c0=0, c1=512):
                eng = nc.sync if d < 3 else nc.scalar
                eng.dma_start(
                    xT[d][:, ch * 512 + c0 : ch * 512 + c1],
                    xT_d[d * 128 : (d + 1) * 128, ch * 512 + c0 : ch * 512 + c1],
                )

            # t=0 microchunks first so project_v(0) starts ASAP
            for d in range(ND):
                load_x(0, d, 0, 128)
            for d in range(ND):
                load_x(0, d, 128, 512)
            nc.scalar.dma_start(wq[:], wq_d[:])
            for d in range(ND):
                load_x(1, d)
            nc.scalar.dma_start(wk[:], wk_d[:])
            for d in range(ND):
                load_x(2, d)
            nc.scalar.dma_start(maskt[:], maskt_d[:])
            for d in range(ND):
                load_x(3, d)
            nc.scalar.dma_start(wo[:], wo_d[:])
            # ones column last: denominator lands at PSUM partition 64 (32-aligned)
            nc.vector.memset(vaug[:, :, :, 64:65], 1.0)

            def project_v(t):
                ps = po_pool.tile([128, E], f32, tag="po", name="psv")
                for d in range(ND):
                    nc.tensor.matmul(
                        ps[:],
                        xT[d][:, t * 128 : (t + 1) * 128],
                        wv[:, d, :],
                        start=(d == 0),
                        stop=(d == ND - 1),
                    )
                if t % 2 == 0:
                    nc.vector.tensor_copy(
                        vaug[:, t, :, 0:64], ps[:].rearrange("p (h e) -> p h e", e=64)
                    )
                else:
                    nc.scalar.activation(
                        vaug[:, t, :, 0:64],
                        ps[:].rearrange("p (h e) -> p h e", e=64),
                        Act.Copy,
                    )

            qT = [None] * NE
            kT = [None] * NE
            attnT = [None] * NE

            def project_chunk(tl, et, w_t, ch):
                ps = po_pool.tile([128, 512], f32, tag="po", name="psp")
                for d in range(ND):
                    nc.tensor.matmul(
                        ps[:],
                        w_t[:, d, et * 128 : (et + 1) * 128],
                        xT[d][:, ch * 512 : (ch + 1) * 512],
                        start=(d == 0),
                        stop=(d == ND - 1),
                    )
                nc.scalar.activation(tl[:, ch * 512 : (ch + 1) * 512], ps[:], Act.Copy)

            def attend2(et, g):
                """Both heads of e-tile et, query group g."""
                ntb = 4 * g + 4
                po = [
                    po_pool.tile([65, 512], f32, tag="po", name=f"po{p}")
                    for p in range(2)
                ]
                pts = []

                def emit_scores(tb):
                    j = tb - 4 * g
                    w0 = 128 * j if j >= 1 else 0
                    ps_s = ps2_pool.tile([128, 2, 512], f32, tag="ps2", name="pss")
                    for p in range(2):
                        po_ = p * 64
                        nc.tensor.matmul(
                            ps_s[:, p, w0:512],
                            kT[et][po_ : po_ + 64, tb * 128 : (tb + 1) * 128],
                            qT[et][po_ : po_ + 64, g * 512 + w0 : (g + 1) * 512],
                            start=True,
                            stop=True,
                        )
                    pt = pt_pool.tile([128, 2, 512], f16, tag="pt")
                    if j >= 0:
                        # diagonal: exact exp on scalar + narrow mask on DVE
                        nc.scalar.activation(
                            pt[:, :, w0:512], ps_s[:, :, w0:512], Act.Exp, scale=0.125
                        )
                        nc.vector.tensor_mul(
                            pt[:, :, w0 : w0 + 128],
                            pt[:, :, w0 : w0 + 128],
                            maskt[:, None, :].to_broadcast((128, 2, 128)),
                        )
                    elif tb % 2 == 1 or _no_sch:
                        nc.scalar.activation(pt[:], ps_s[:], Act.Exp, scale=0.125)
                    else:
                        # Schraudolph fp16-bit exp in one DVE op
                        nc.vector.tensor_scalar(
                            pt[:].bitcast(i16),
                            ps_s[:],
                            SCH_A,
                            SCH_B,
                            Alu.mult,
                            Alu.add,
                        )
                    pts.append((pt, w0))

                def emit_out(tb):
                    pt, w0 = pts[tb]
                    for p in range(2):
                        h = 2 * et + p
                        nc.tensor.matmul(
                            po[p][:, w0:512],
                            vaug[:, tb, h, :],  # col 64 = ones -> denom row 64
                            pt[:, p, w0:512],
                            start=(tb == 0),
                            stop=(tb == ntb - 1),
                        )

                emitted = 0
                for tb in range(ntb):
                    emit_scores(tb)
                    if tb >= 2:
                        emit_out(emitted)
                        emitted += 1
                while emitted < ntb:
                    emit_out(emitted)
                    emitted += 1

                # epilogue: denominators (PSUM partition 0) -> SBUF -> gpsimd
                # partition-broadcast halves -> one full-width reciprocal ->
                # fused copy*normalize into attnT
                # two independent per-head chains: dn copy -> broadcast raw
                # denom -> reciprocal on the half -> fused normalize
                dn = dn_pool.tile([1, 2, 512], f32, tag="dn", name="dn")
                rb = [
                    rbs_pool.tile([128, 512], f32, tag="rbp", name=f"rb{p}")
                    for p in range(2)
                ]
                nc.scalar.activation(dn[0:1, 0, :], po[0][64:65, :], Act.Copy)
                nc.vector.tensor_copy(dn[0:1, 1, :], po[1][64:65, :])
                dnr = dn_pool.tile([1, 2, 512], f32, tag="dn", name="dnr")
                nc.vector.reciprocal_approx_fast(dnr[:], dn[:])
                for p in range(2):
                    nc.gpsimd.partition_broadcast(rb[p][:], dnr[0:1, p, :])
                for p in range(2):
                    nc.vector.scalar_tensor_tensor(
                        attnT[et][p * 64 : (p + 1) * 64, g * 512 : (g + 1) * 512],
                        po[p][0:64, :],
                        1.0,
                        rb[p][p * 64 : (p + 1) * 64, :],
                        Alu.mult,
                        Alu.mult,
                    )

            for et in range(NE):
                attnT[et] = at_pool.tile([128, S], f16, tag="at", name=f"attnT{et}")

            def wo_chain(t):
                ysb = y_pool.tile([128, 2, 384], f16, tag="y")
                for ch in range(2):
                    psw = po_pool.tile([128, 384], f32, tag="po", name="psw")
                    for e in range(NE):
                        nc.tensor.matmul(
                            psw[:],
                            attnT[e][:, t * 128 : (t + 1) * 128],
                            wo[:, e, ch * 384 : (ch + 1) * 384],
                            start=(e == 0),
                            stop=(e == NE - 1),
                        )
                    if (t + ch) % 2 == 0:
                        nc.scalar.activation(ysb[:, ch, :], psw[:], Act.Copy)
                    else:
                        nc.vector.tensor_copy(ysb[:, ch, :], psw[:])
                nc.sync.dma_start(y_d[t * 128 : (t + 1) * 128, :], ysb[:])

            for et in range(NE):
                qT[et] = qk_pool.tile([128, S], f16, tag="qk", name=f"qT{et}")
                kT[et] = qk_pool.tile([128, S], f16, tag="qk", name=f"kT{et}")

            # group g's projections (x chunk g) are emitted right before its
            # attention; Wo chains for group g-1 are staggered into group g's
            # et boundaries to fill tensor-engine gaps
            pending_wo = []
            for g in range(NG):
                for t in range(4 * g, 4 * g + 4):
                    project_v(t)
                for et in range(NE):
                    project_chunk(qT[et], et, wq, g)
                    project_chunk(kT[et], et, wk, g)
                for et in range(NE):
                    attend2(et, g)
                    if pending_wo:
                        wo_chain(pending_wo.pop(0))
                while pending_wo:
                    wo_chain(pending_wo.pop(0))
                pending_wo = list(range(4 * g, 4 * g + 4))
            for t in pending_wo:
                wo_chain(t)

    nc.compile()
    return nc


def _in_maps(x, Wq, Wk, Wv, Wo):
    tt = np.arange(128)[:, None]
    qq = np.arange(128)[None, :]
    maskt = (tt <= qq).astype(np.float16)

    def wrearr(wT, n):  # [768 or 384, out] -> [128, n, out]
        return np.ascontiguousarray(
            wT.reshape(n, 128, wT.shape[1]).transpose(1, 0, 2)
        ).astype(np.float16)

    maps = []
    for c in range(8):
        b, hg = c // 2, c % 2
        rows = slice(hg * E, (hg + 1) * E)
        maps.append(
            {
                "xT": np.ascontiguousarray(x[:, b, :].T).astype(np.float16),
                "wq": wrearr(np.ascontiguousarray(Wq[rows].T).astype(np.float32), ND),
                "wk": wrearr(np.ascontiguousarray(Wk[rows].T).astype(np.float32), ND),
                "wv": wrearr(np.ascontiguousarray(Wv[rows].T).astype(np.float32), ND),
                "wo": wrearr(np.ascontiguousarray(Wo[:, rows].T).astype(np.float32), NE),
                "maskt": maskt,
            }
        )
    return maps


def get_nc():
    global _cached
    if _cached is None:
        _cached = _build()
    return _cached


def kernel(x, Wq, Wk, Wv, Wo):
    from concourse.bass_utils import run_bass_kernel_spmd

    x = np.asarray(x, dtype=np.float32)
    nc = get_nc()
    in_maps = _in_maps(x, Wq, Wk, Wv, Wo)
    last_err = None
    for _attempt in range(3):
        try:
            res = run_bass_kernel_spmd(nc, in_maps, core_ids=list(range(8)))
            break
        except Exception as e:  # transient NRT device errors: retry
            last_err = e
    else:
        raise last_err
    out = np.empty((S, B, D), dtype=np.float32)
    for b in range(B):
        out[:, b, :] = res.results[2 * b]["y"].astype(np.float32) + res.results[
            2 * b + 1
        ]["y"].astype(np.float32)
    return out


# revision 54
# speedup vs baseline: 1.0173x; 1.0173x over previous
"""Causal self-attention (S=2048, B=4, D=768, H=12, Hd=64) on 8 TRN2 cores.

Sharding: core c -> (batch b = c//2, head-group hg = c%2).  Each core computes
full-seq attention for one batch element and 6 of the 12 heads plus the Wo
projection restricted to its heads' columns; the host sums the two head-group
partials per batch (fp16 partials -> fp32 on host).

v2 design (vs baseline):
  - g-major loop: per query-group g (512 q), attend over all 3 head-pair
    tiles, then normalize + Wo-project + store that group's y while group
    g+1 attends.  No end-of-kernel stall.
  - softmax exp split across engines: diagonal blocks exact exp on Scalar
    (+ narrow [128]-col mask multiply on GpSimd); off-diagonal blocks split
    between DVE (one-op Schraudolph fp16-bit exp via tensor_scalar ->int16)
    and Scalar.
  - deferred normalization: denominators ride the AV matmul (ones column,
    M=65); per (et,g): 2 scalar copies -> reciprocal_approx_fast (DVE) ->
    K=2 broadcast matmul (f32r) -> fused copy*normalize into attnT (DVE).
  - host pre-rearranged weights (contiguous DMA), x on sync queue, weights
    on scalar queue, fp16 y output.
"""

import numpy as np

S = 2048
B = 4
D = 768
HD = 64
H = 6          # heads per core
E = H * HD     # 384
ND = D // 128  # 6
NE = E // 128  # 3
NT = S // 128  # 16
NG = S // 512  # 4

# Schraudolph fp16-bits exp: bits16 = round(raw_score*A + Bc); see exp_precision.py
SCH_A = 184.66494318  # 0.125 * 1024 / ln(2)
SCH_B = 15316.0       # 15360 + c_adj (c_adj=-44)

_cached = None


def _build():
    import os

    import concourse.mybir as mybir
    import concourse.tile as tile
    from concourse import bacc

    f32 = mybir.dt.float32
    f32r = mybir.dt.float32r
    f16 = mybir.dt.float16
    i16 = mybir.dt.int16
    Alu = mybir.AluOpType
    Act = mybir.ActivationFunctionType

    _no_sch = os.environ.get("K_NO_SCH", "0") == "1"
    _no_fused = os.environ.get("K_NO_FUSED", "0") == "1"

    nc = bacc.Bacc("TRN2")

    xT_d = nc.dram_tensor("xT", [D, S], f16, kind="ExternalInput")
    wq_d = nc.dram_tensor("wq", [128, ND, E], f16, kind="ExternalInput")
    wk_d = nc.dram_tensor("wk", [128, ND, E], f16, kind="ExternalInput")
    wv_d = nc.dram_tensor("wv", [128, ND, E], f16, kind="ExternalInput")
    wo_d = nc.dram_tensor("wo", [128, NE, D], f16, kind="ExternalInput")
    maskt_d = nc.dram_tensor("maskt", [128, 128], f16, kind="ExternalInput")
    y_d = nc.dram_tensor("y", [S, D], f16, kind="ExternalOutput")

    with tile.TileContext(nc) as tc:
        with (
            tc.tile_pool(name="xt", bufs=6) as xt_pool,
            tc.tile_pool(name="at", bufs=3) as at_pool,
            tc.tile_pool(name="w", bufs=1) as w_pool,
            tc.tile_pool(name="qk", bufs=6) as qk_pool,
            tc.tile_pool(name="pt", bufs=6) as pt_pool,
            tc.tile_pool(name="dn", bufs=6) as dn_pool,
            tc.tile_pool(name="rbp", bufs=6) as rbs_pool,
            tc.tile_pool(name="y", bufs=4) as y_pool,
            tc.tile_pool(name="ps2", bufs=2, space="PSUM") as ps2_pool,
            tc.tile_pool(name="po", bufs=4, space="PSUM") as po_pool,
        ):
            wq = w_pool.tile([128, ND, E], f16, tag="wq")
            wk = w_pool.tile([128, ND, E], f16, tag="wk")
            wv = w_pool.tile([128, ND, E], f16, tag="wv")
            wo = w_pool.tile([128, NE, D], f16, tag="wo")
            maskt = w_pool.tile([128, 128], f16, tag="maskt")
            vaug = w_pool.tile([128, NT, H, 65], f16, tag="vaug")

            # x chunks split across both DMA queues (ch-major so t=0..3 land
            # first); wv leads the scalar queue so project_v can start early
            xT = [
                xt_pool.tile([128, S], f16, tag="xt", name=f"xT{d}")
                for d in range(ND)
            ]
            nc.sync.dma_start(wv[:, 0:3, :], wv_d[:, 0:3, :])
            nc.scalar.dma_start(wv[:, 3:6, :], wv_d[:, 3:6, :])

            def load_x(ch, d, c0=0, c1=512):
                eng = nc.sync if d < 3 else nc.scalar
                eng.dma_start(
                    xT[d][:, ch * 512 + c0 : ch * 512 + c1],
                    xT_d[d * 128 : (d + 1) * 128, ch * 512 + c0 : ch * 512 + c1],
                )

            # t=0 microchunks first so project_v(0) starts ASAP
            for d in range(ND):
                load_x(0, d, 0, 128)
            for d in range(ND):
                load_x(0, d, 128, 512)
            nc.scalar.dma_start(wq[:], wq_d[:])
            for d in range(ND):
                load_x(1, d)
            nc.scalar.dma_start(wk[:], wk_d[:])
            for d in range(ND):
                load_x(2, d)
            nc.scalar.dma_start(maskt[:], maskt_d[:])
            for d in range(ND):
                load_x(3, d)
            nc.scalar.dma_start(wo[:], wo_d[:])
            # ones column last: denominator lands at PSUM partition 64 (32-aligned)
            nc.vector.memset(vaug[:, :, :, 64:65], 1.0)

            # ---- v projection ----
            for t in range(NT):
                ps = po_pool.tile([128, E], f32, tag="po", name="psv")
                for d in range(ND):
                    nc.tensor.matmul(
                        ps[:],
                        xT[d][:, t * 128 : (t + 1) * 128],
                        wv[:, d, :],
                        start=(d == 0),
                        stop=(d == ND - 1),
                    )
                if t % 2 == 0:
                    nc.vector.tensor_copy(
                        vaug[:, t, :, 0:64], ps[:].rearrange("p (h e) -> p h e", e=64)
                    )
                else:
                    nc.scalar.activation(
                        vaug[:, t, :, 0:64],
                        ps[:].rearrange("p (h e) -> p h e", e=64),
                        Act.Copy,
                    )

            qT = [None] * NE
            kT = [None] * NE
            attnT = [None] * NE

            def project(dst_list, et, w_t, nm):
                tl = qk_pool.tile([128, S], f16, tag="qk", name=f"{nm}T{et}")
                for ch in range(4):
                    ps = po_pool.tile([128, 512], f32, tag="po", name="psp")
                    for d in range(ND):
                        nc.tensor.matmul(
                            ps[:],
                            w_t[:, d, et * 128 : (et + 1) * 128],
                            xT[d][:, ch * 512 : (ch + 1) * 512],
                            start=(d == 0),
                            stop=(d == ND - 1),
                        )
                    nc.scalar.activation(tl[:, ch * 512 : (ch + 1) * 512], ps[:], Act.Copy)
                dst_list[et] = tl

            def attend2(et, g):
                """Both heads of e-tile et, query group g."""
                ntb = 4 * g + 4
                po = [
                    po_pool.tile([65, 512], f32, tag="po", name=f"po{p}")
                    for p in range(2)
                ]
                pts = []

                def emit_scores(tb):
                    j = tb - 4 * g
                    w0 = 128 * j if j >= 1 else 0
                    ps_s = ps2_pool.tile([128, 2, 512], f32, tag="ps2", name="pss")
                    for p in range(2):
                        po_ = p * 64
                        nc.tensor.matmul(
                            ps_s[:, p, w0:512],
                            kT[et][po_ : po_ + 64, tb * 128 : (tb + 1) * 128],
                            qT[et][po_ : po_ + 64, g * 512 + w0 : (g + 1) * 512],
                            start=True,
                            stop=True,
                        )
                    pt = pt_pool.tile([128, 2, 512], f16, tag="pt")
                    if j >= 0:
                        # diagonal: exact exp on scalar + narrow mask on DVE
                        nc.scalar.activation(
                            pt[:, :, w0:512], ps_s[:, :, w0:512], Act.Exp, scale=0.125
                        )
                        nc.vector.tensor_mul(
                            pt[:, :, w0 : w0 + 128],
                            pt[:, :, w0 : w0 + 128],
                            maskt[:, None, :].to_broadcast((128, 2, 128)),
                        )
                    elif tb % 2 == 1 or _no_sch:
                        nc.scalar.activation(pt[:], ps_s[:], Act.Exp, scale=0.125)
                    else:
                        # Schraudolph fp16-bit exp in one DVE op
                        nc.vector.tensor_scalar(
                            pt[:].bitcast(i16),
                            ps_s[:],
                            SCH_A,
                            SCH_B,
                            Alu.mult,
                            Alu.add,
                        )
                    pts.append((pt, w0))

                def emit_out(tb):
                    pt, w0 = pts[tb]
                    for p in range(2):
                        h = 2 * et + p
                        nc.tensor.matmul(
                            po[p][:, w0:512],
                            vaug[:, tb, h, :],  # col 64 = ones -> denom row 64
                            pt[:, p, w0:512],
                            start=(tb == 0),
                            stop=(tb == ntb - 1),
                        )

                emitted = 0
                for tb in range(ntb):
                    emit_scores(tb)
                    if tb >= 2:
                        emit_out(emitted)
                        emitted += 1
                while emitted < ntb:
                    emit_out(emitted)
                    emitted += 1

                # epilogue: denominators (PSUM partition 0) -> SBUF -> gpsimd
                # partition-broadcast halves -> one full-width reciprocal ->
                # fused copy*normalize into attnT
                # two independent per-head chains: dn copy -> broadcast raw
                # denom -> reciprocal on the half -> fused normalize
                dn = dn_pool.tile([1, 2, 512], f32, tag="dn", name="dn")
                rb = [
                    rbs_pool.tile([128, 512], f32, tag="rbp", name=f"rb{p}")
                    for p in range(2)
                ]
                nc.scalar.activation(dn[0:1, 0, :], po[0][64:65, :], Act.Copy)
                nc.vector.tensor_copy(dn[0:1, 1, :], po[1][64:65, :])
                dnr = dn_pool.tile([1, 2, 512], f32, tag="dn", name="dnr")
                nc.vector.reciprocal_approx_fast(dnr[:], dn[:])
                for p in range(2):
                    nc.gpsimd.partition_broadcast(rb[p][:], dnr[0:1, p, :])
                for p in range(2):
                    nc.vector.scalar_tensor_tensor(
                        attnT[et][p * 64 : (p + 1) * 64, g * 512 : (g + 1) * 512],
                        po[p][0:64, :],
                        1.0,
                        rb[p][p * 64 : (p + 1) * 64, :],
                        Alu.mult,
                        Alu.mult,
                    )

            for et in range(NE):
                attnT[et] = at_pool.tile([128, S], f16, tag="at", name=f"attnT{et}")

            def wo_chain(t):
                ysb = y_pool.tile([128, 2, 384], f16, tag="y")
                for ch in range(2):
                    psw = po_pool.tile([128, 384], f32, tag="po", name="psw")
                    for e in range(NE):
                        nc.tensor.matmul(
                            psw[:],
                            attnT[e][:, t * 128 : (t + 1) * 128],
                            wo[:, e, ch * 384 : (ch + 1) * 384],
                            start=(e == 0),
                            stop=(e == NE - 1),
                        )
                    if (t + ch) % 2 == 0:
                        nc.scalar.activation(ysb[:, ch, :], psw[:], Act.Copy)
                    else:
                        nc.vector.tensor_copy(ysb[:, ch, :], psw[:])
                nc.sync.dma_start(y_d[t * 128 : (t + 1) * 128, :], ysb[:])

            # Wo chains for group g-1 are staggered into group g's et
            # boundaries to fill tensor-engine gaps
            pending_wo = []
            for g in range(NG):
                for et in range(NE):
                    if g == 0:
                        project(qT, et, wq, "q")
                        project(kT, et, wk, "k")
                    attend2(et, g)
                    if pending_wo:
                        wo_chain(pending_wo.pop(0))
                while pending_wo:
                    wo_chain(pending_wo.pop(0))
                pending_wo = list(range(4 * g, 4 * g + 4))
            for t in pending_wo:
                wo_chain(t)

    nc.compile()
    return nc


def _in_maps(x, Wq, Wk, Wv, Wo):
    tt = np.arange(128)[:, None]
    qq = np.arange(128)[None, :]
    maskt = (tt <= qq).astype(np.float16)

    def wrearr(wT, n):  # [768 or 384, out] -> [128, n, out]
        return np.ascontiguousarray(
            wT.reshape(n, 128, wT.shape[1]).transpose(1, 0, 2)
        ).astype(np.float16)

    maps = []
    for c in range(8):
        b, hg = c // 2, c % 2
        rows = slice(hg * E, (hg + 1) * E)
        maps.append(
            {
                "xT": np.ascontiguousarray(x[:, b, :].T).astype(np.float16),
                "wq": wrearr(np.ascontiguousarray(Wq[rows].T).astype(np.float32), ND),
                "wk": wrearr(np.ascontiguousarray(Wk[rows].T).astype(np.float32), ND),
                "wv": wrearr(np.ascontiguousarray(Wv[rows].T).astype(np.float32), ND),
                "wo": wrearr(np.ascontiguousarray(Wo[:, rows].T).astype(np.float32), NE),
                "maskt": maskt,
            }
        )
    return maps


def get_nc():
    global _cached
    if _cached is None:
        _cached = _build()
    return _cached


def kernel(x, Wq, Wk, Wv, Wo):
    from concourse.bass_utils import run_bass_kernel_spmd

    x = np.asarray(x, dtype=np.float32)
    nc = get_nc()
    in_maps = _in_maps(x, Wq, Wk, Wv, Wo)
    last_err = None
    for _attempt in range(3):
        try:
            res = run_bass_kernel_spmd(nc, in_maps, core_ids=list(range(8)))
            break
        except Exception as e:  # transient NRT device errors: retry
            last_err = e
    else:
        raise last_err
    out = np.empty((S, B, D), dtype=np.float32)
    for b in range(B):
        out[:, b, :] = res.results[2 * b]["y"].astype(np.float32) + res.results[
            2 * b + 1
        ]["y"].astype(np.float32)
    return out
